# revision 3
# baseline (speedup 1.0000x reference)
"""Causal single-head attention on 8 Trainium2 NeuronCores, K/V pair-exchange.

Problem: x [4, 2048, 1024], w_q/w_k/w_v [1024, 1024] (nn.Linear convention,
y = x @ W.T). Computes q,k,v projections, causal softmax(q k^T / sqrt(D)) @ v.

Sharding: 2 cores per batch element. Core parity p owns token half
H_p = [p*1024, (p+1)*1024) and computes K^T/V for ONLY its half; halves are
exchanged between the pair via four 1MB AllGathers (replica groups [[0,1],
[2,3],[4,5],[6,7]]): K tokens [0:512] of each half, K tokens [512:1024],
V tokens [0:512], V [512:1024]. The two V AGs are fenced behind the two K
AGs by data-dependency rows in their bounce buffers (2 concurrent pair-AGs
are safe, 3+ corrupt the odd member - measured in a previous session).
Queries: parity-interleaved 128-tiles (slot k has a kv window of 256k
tokens), host-gathered, projected to Q^T kept in SBUF. Q^T is projected
LAST so it fills the PE while the K AGs are in flight.

Attention computes S^T = K^T-chunk^T Q (scores transposed, token-major) so
softmax exp output IS the P^T layout that the AV matmul needs - no PE
transposes, no PSUM round trips. exp reads straight from PSUM (no max
subtraction - scores*1/sqrt(D) are O(1) so exp cannot overflow); row sums
come from an accumulating ones-matmul; the causal mask is folded in as one
extra accumulation matmul (identity @ maskT) per boundary chunk. AV runs as
two passes: first chunks carried by the V-a AllGather, then V-b chunks, so
the last AG can arrive late without stalling the PE queue.

All matmul operands are bf16; softmax statistics and PSUM stay f32. Large
DMAs are batched into single strided transfers (a DMA trigger costs ~600ns
of engine queue time).
"""
import numpy as np
import ml_dtypes
from contextlib import ExitStack

import concourse.bass as bass
import concourse.tile as tile
import concourse.mybir as mybir
from concourse.bass_utils import run_bass_kernel_spmd
from concourse.masks import make_identity

F32 = mybir.dt.float32
BF16 = mybir.dt.bfloat16
AF = mybir.ActivationFunctionType
AX = mybir.AxisListType

B, S, E, D = 4, 2048, 1024, 1024
NCORES = 8
NSLOT = 8              # slots k=1..8, kv window = 256*k tokens
NQ = NSLOT * 128       # queries per core
HT = S // 2            # tokens projected per core (own half)
HH = HT // 2           # token quarter (AG granularity)
EC = E // 128          # e-chunks
DC = D // 128          # d-chunks
NCH = S // 128         # token chunks
SCALE = 1.0 / 32.0     # 1/sqrt(D)
MASKVAL = -30000.0
GROUPS = [[0, 1], [2, 3], [4, 5], [6, 7]]

_prog = None


def _split_multi_waits(nc, max_waits=1):
    """The walrus build in this container has one sync-wait slot per
    instruction; hoist extra waits onto preceding same-engine NoOps."""
    n = 0
    for f in nc.m.functions:
        for b in f.blocks:
            insts = b.instructions
            out = []
            changed = False
            for ins in insts:
                si = ins.sync_info
                if si is not None and len(si.on_wait) > max_waits:
                    waits = list(si.on_wait)
                    for w in waits[:-max_waits]:
                        nop = mybir.InstNoOp(name=f"I-waitsplit-{n}")
                        n += 1
                        nop.engine = ins.engine
                        nop.sync_info = mybir.SyncInfo(on_wait=[w], on_update=[])
                        out.append(nop)
                    ins.sync_info = mybir.SyncInfo(
                        on_wait=waits[-max_waits:], on_update=list(si.on_update))
                    changed = True
                out.append(ins)
            if changed:
                b.instructions = out
    return nc


def _build(split=True):
    nc = bass.Bass(trn_type="TRN2", target_bir_lowering=False, debug=False)
    xoT = nc.dram_tensor("xoT", [E, HT], BF16, kind="ExternalInput").ap()
    xqT = nc.dram_tensor("xqT", [E, NQ], BF16, kind="ExternalInput").ap()
    wqT = nc.dram_tensor("wqT", [E, D], BF16, kind="ExternalInput").ap()
    wkT = nc.dram_tensor("wkT", [E, D], BF16, kind="ExternalInput").ap()
    wvT = nc.dram_tensor("wvT", [E, D], BF16, kind="ExternalInput").ap()
    # maskT: transposed causal boundary mask [2*128 window rows, 128 queries]
    maskin = nc.dram_tensor("maskT", [256, 128], BF16, kind="ExternalInput").ap()
    onesin = nc.dram_tensor("ones", [128, 1], BF16, kind="ExternalInput").ap()
    out = nc.dram_tensor("out", [NQ, D], F32, kind="ExternalOutput").ap()

    # collective bounce/gather buffers. K^T split by token half; V split by
    # token half with a fence row carrying K-AG output bytes so at most two
    # pair-AGs are ever in flight.
    bncK, gathK = [], []
    for g in range(2):
        bncK.append(nc.dram_tensor(f"bncK{g}", [DC, 128, HH], BF16).ap())
        gathK.append(nc.dram_tensor(f"gathK{g}", [2, DC, 128, HH], BF16).ap())
    bncV, gathV = [], []
    for v in range(2):
        bncV.append(nc.dram_tensor(f"bncV{v}", [HH + 1, D], BF16).ap())
        gathV.append(nc.dram_tensor(f"gathV{v}", [2, HH + 1, D], BF16).ap())

    with tile.TileContext(nc) as tc, ExitStack() as ctx:
        const = ctx.enter_context(tc.tile_pool(name="const", bufs=1))
        ident = const.tile([128, 128], BF16)
        make_identity(nc, ident[:])
        maskT = const.tile([128, 256], BF16)   # [:, 0:128]=rows 0:128, etc
        nc.sync.dma_start(maskT[:, 0:128], maskin[0:128, :])
        nc.sync.dma_start(maskT[:, 128:256], maskin[128:256, :])
        ones = const.tile([128, 1], BF16)
        nc.sync.dma_start(ones[:], onesin[:])

        # Q^T stays resident until the end of attention
        qtp = ctx.enter_context(tc.tile_pool(name="qtp", bufs=1))
        qts = qtp.tile([128, DC * NQ], BF16, name="qts")   # col d*NQ + q

        # ---- Phase 1: K_own^T -> AGs (by token half), V_own -> AGs, Q^T ----
        with tc.tile_pool(name="wp", bufs=1) as wp, \
             tc.tile_pool(name="xp", bufs=1) as xp, \
             tc.tile_pool(name="st", bufs=1) as stp, \
             tc.tile_pool(name="ps1", bufs=4, space="PSUM") as pp:
            # fused weight/activation tiles: col = e*1024 + c
            wk = wp.tile([128, EC * D], BF16, name="wk")
            wv = wp.tile([128, EC * D], BF16, name="wv")
            wq = wp.tile([128, EC * D], BF16, name="wq")
            xo = xp.tile([128, EC * HT], BF16, name="xo")
            xq = xp.tile([128, EC * NQ], BF16, name="xq")

            def eview(t, w):       # [128, EC*w] -> [128, EC, w]
                return t[:].rearrange("p (e c) -> p e c", e=EC)

            wkT3 = wkT.rearrange("(e p) c -> p e c", p=128)
            nc.sync.dma_start(eview(wk, D)[:, :, 0:512], wkT3[:, :, 0:512])
            xoT3 = xoT.rearrange("(e p) c -> p e c", p=128)
            nc.sync.dma_start(eview(xo, HT)[:, :, 0:512], xoT3[:, :, 0:512])
            nc.sync.dma_start(eview(xo, HT)[:, :, 512:HT], xoT3[:, :, 512:HT])
            nc.sync.dma_start(eview(wk, D)[:, :, 512:D], wkT3[:, :, 512:D])
            nc.sync.dma_start(eview(wv, D)[:], wvT.rearrange("(e p) c -> p e c", p=128))
            nc.sync.dma_start(eview(wq, D)[:], wqT.rearrange("(e p) c -> p e c", p=128))
            nc.sync.dma_start(eview(xq, NQ)[:], xqT.rearrange("(e p) c -> p e c", p=128))

            # K_own^T: token-half-outer so AG-K0 (first 512 own tokens of the
            # pair) triggers after half of K_own. kown col = dd*HT + t.
            kown = stp.tile([128, DC * HT], BF16, name="kown")
            for g in range(2):
                for q in range(2):
                    psl = [pp.tile([128, 512], F32, name=f"pk{g}_{q}_{d}",
                                   tag="pp") for d in range(4)]
                    for e in range(EC):
                        for d in range(4):
                            dd = q * 4 + d
                            nc.tensor.matmul(
                                psl[d][:],
                                wk[:, e * D + dd * 128:e * D + (dd + 1) * 128],
                                xo[:, e * HT + g * 512:e * HT + (g + 1) * 512],
                                start=(e == 0), stop=(e == EC - 1))
                    for d in range(4):
                        dd = q * 4 + d
                        nc.vector.tensor_copy(
                            kown[:, dd * HT + g * 512:dd * HT + (g + 1) * 512],
                            psl[d][:])
                    # one strided bounce DMA for the whole (g, q) quadrant
                    ksrc = kown[:].rearrange("p (d t) -> p d t", d=DC)
                    nc.scalar.dma_start(
                        bncK[g][q * 4:(q + 1) * 4].rearrange("d p t -> p d t"),
                        ksrc[:, q * 4:(q + 1) * 4, g * 512:(g + 1) * 512])
                nc.gpsimd.collective_compute(
                    "AllGather", mybir.AluOpType.bypass, replica_groups=GROUPS,
                    ins=[bncK[g].opt()], outs=[gathK[g].opt()])

            # V_own: stationary x chunks, moving wv; token quarter v first.
            # vown col = t*D + c  (t = own-half token chunk 0..7)
            vown = stp.tile([128, (HT // 128) * D], BF16, name="vown")
            for v in range(2):
                for tl in range(HH // 128):
                    t = v * (HH // 128) + tl
                    for h in range(2):
                        ps = pp.tile([128, 512], F32, name=f"pv{t}_{h}", tag="pp")
                        for e in range(EC):
                            nc.tensor.matmul(
                                ps[:],
                                xo[:, e * HT + t * 128:e * HT + (t + 1) * 128],
                                wv[:, e * D + h * 512:e * D + (h + 1) * 512],
                                start=(e == 0), stop=(e == EC - 1))
                        nc.vector.tensor_copy(
                            vown[:, t * D + h * 512:t * D + (h + 1) * 512],
                            ps[:])
                vsrc = vown[:].rearrange("p (t c) -> p t c", t=HT // 128)
                nc.scalar.dma_start(
                    bncV[v][0:HH].rearrange("(t p) c -> p t c", p=128),
                    vsrc[:, v * 4:(v + 1) * 4, :])
                # fence: the V AG may only trigger once the same-index K AG
                # has fully delivered (reads replica-1 bytes of its output)
                nc.scalar.dma_start(bncV[v][HH:HH + 1, 0:16],
                                    gathK[v][1, 0, 0:1, 0:16])
                nc.gpsimd.collective_compute(
                    "AllGather", mybir.AluOpType.bypass, replica_groups=GROUPS,
                    ins=[bncV[v].opt()], outs=[gathV[v].opt()])

            # Q^T last: fills the PE while the K AGs fly. col = d*NQ + q.
            for d in range(DC):
                for g in range(2):
                    ps = pp.tile([128, 512], F32, name=f"pq{d}_{g}", tag="pp")
                    for e in range(EC):
                        nc.tensor.matmul(
                            ps[:],
                            wq[:, e * D + d * 128:e * D + (d + 1) * 128],
                            xq[:, e * NQ + g * 512:e * NQ + (g + 1) * 512],
                            start=(e == 0), stop=(e == EC - 1))
                    nc.vector.tensor_copy(
                        qts[:, d * NQ + g * 512:d * NQ + (g + 1) * 512], ps[:])

        # ---- Phase 2: load gathered K^T / V into SBUF (batched DMAs) ----
        # kts col = d*S + t (global token order); vts col = t*D + c
        kvp = ctx.enter_context(tc.tile_pool(name="kvp", bufs=1))
        kts = kvp.tile([128, DC * S], BF16, name="kts")
        vts = kvp.tile([128, NCH * D], BF16, name="vts")
        ktsv = kts[:].rearrange("p (d t) -> p d t", d=DC)
        for g in range(2):
            for r in range(2):
                base = r * HT + g * HH
                nc.sync.dma_start(ktsv[:, :, base:base + HH],
                                  gathK[g][:, :, :, :][r].rearrange("d p t -> p d t"))
        vtsv = vts[:].rearrange("p (t c) -> p t c", t=NCH)
        for v in range(2):
            for r in range(2):
                tb = r * 8 + v * 4   # first global chunk of this quarter
                nc.sync.dma_start(vtsv[:, tb:tb + 4, :],
                                  gathV[v][r, 0:HH].rearrange("(t p) c -> p t c", p=128))

        # ---- Phase 3: S^T scores + softmax (P^T straight out) per slot ----
        att = ctx.enter_context(tc.tile_pool(name="att", bufs=1))
        stats = ctx.enter_context(tc.tile_pool(name="stats", bufs=1))
        linv = stats.tile([128, NSLOT], F32, name="linv")
        pT = {k: att.tile([128, 256 * k], BF16, name=f"pT{k}")
              for k in range(1, NSLOT + 1)}
        # per-slot unnormalized O from the V-a chunks, awaiting the V-b part
        osb = {k: att.tile([128, D], F32, name=f"osb{k}")
               for k in range(3, NSLOT + 1)}
        av_a = {k: [c for c in range(2 * k) if c % 8 < 4]
                for k in range(1, NSLOT + 1)}
        av_b = {k: [c for c in range(2 * k) if c % 8 >= 4]
                for k in range(1, NSLOT + 1)}

        with tc.tile_pool(name="ps3", bufs=1, space="PSUM") as pp3:
            pending_lsum = []

            def emit_lsum(k):
                nch = 2 * k
                ls = pp3.tile([128, 2], F32, name=f"ls{k}", tag="lsp", bufs=2)
                for c in range(nch):
                    nc.tensor.matmul(ls[:, 0:1], pT[k][:, c * 128:(c + 1) * 128],
                                     ones[:], start=(c == 0), stop=(c == nch - 1))
                nc.vector.reciprocal(linv[:, k - 1:k], ls[:, 0:1])

            for k in range(1, NSLOT + 1):
                nch = 2 * k
                ngrp = (nch + 3) // 4
                qsl = qts[:]  # noqa
                sT = [pp3.tile([128, 512], F32, name=f"sT{k}_{g}", tag="sps",
                               bufs=4) for g in range(ngrp)]
                for c in range(nch):
                    g, j = divmod(c, 4)
                    dst = sT[g][:, j * 128:(j + 1) * 128]
                    masked = c >= nch - 2
                    for d in range(DC):
                        nc.tensor.matmul(
                            dst,
                            kts[:, d * S + c * 128:d * S + (c + 1) * 128],
                            qts[:, d * NQ + (k - 1) * 128:d * NQ + k * 128],
                            start=(d == 0), stop=(d == DC - 1 and not masked))
                    if masked:
                        mo = 0 if c == nch - 2 else 128
                        nc.tensor.matmul(dst, ident[:], maskT[:, mo:mo + 128],
                                         start=False, stop=True,
                                         skip_group_check=True)
                # exp straight from PSUM; output IS P^T (token-major)
                for g in range(ngrp):
                    w = min(512, nch * 128 - g * 512)
                    nc.scalar.activation(pT[k][:, g * 512:g * 512 + w],
                                         sT[g][:, :w], AF.Exp, scale=SCALE)
                # one-slot lag on the row-sum matmuls so the PE never waits
                # on the exp of the slot it just scored
                if pending_lsum:
                    emit_lsum(pending_lsum.pop())
                pending_lsum.append(k)
            emit_lsum(pending_lsum.pop())

            # ---- Phase 4a: AV over the V-a chunks for every slot ----
            for k in range(1, NSLOT + 1):
                ca = av_a[k]
                o_ps = [pp3.tile([128, 512], F32, name=f"oa{k}_{h}", tag="ops",
                                 bufs=2) for h in range(2)]
                for ci, c in enumerate(ca):
                    for h in range(2):
                        nc.tensor.matmul(o_ps[h][:],
                                         pT[k][:, c * 128:(c + 1) * 128],
                                         vts[:, c * D + h * 512:c * D + (h + 1) * 512],
                                         start=(ci == 0), stop=(ci == len(ca) - 1))
                if not av_b[k]:
                    # k=1,2: fully V-a resident; normalize and write out
                    o_fin = att.tile([128, D], F32, name=f"ofa{k}", tag="ofin",
                                     bufs=2)
                    for h in range(2):
                        nc.scalar.activation(o_fin[:, h * 512:(h + 1) * 512],
                                             o_ps[h][:], AF.Copy,
                                             scale=linv[:, k - 1:k])
                    nc.sync.dma_start(out[(k - 1) * 128:k * 128, :], o_fin[:])
                else:
                    # stash normalized partial; V-b part added in phase 4b
                    for h in range(2):
                        nc.scalar.activation(osb[k][:, h * 512:(h + 1) * 512],
                                             o_ps[h][:], AF.Copy,
                                             scale=linv[:, k - 1:k])

            # ---- Phase 4b: AV over the V-b chunks, add, write out ----
            for k in range(3, NSLOT + 1):
                cb = av_b[k]
                o_ps = [pp3.tile([128, 512], F32, name=f"ob{k}_{h}", tag="ops",
                                 bufs=2) for h in range(2)]
                for ci, c in enumerate(cb):
                    for h in range(2):
                        nc.tensor.matmul(o_ps[h][:],
                                         pT[k][:, c * 128:(c + 1) * 128],
                                         vts[:, c * D + h * 512:c * D + (h + 1) * 512],
                                         start=(ci == 0), stop=(ci == len(cb) - 1))
                o_fin = att.tile([128, D], F32, name=f"ofb{k}", tag="ofin",
                                 bufs=2)
                o_sc = att.tile([128, D], F32, name=f"osc{k}", tag="osc", bufs=2)
                for h in range(2):
                    hs = slice(h * 512, (h + 1) * 512)
                    nc.scalar.activation(o_sc[:, hs], o_ps[h][:], AF.Copy,
                                         scale=linv[:, k - 1:k])
                    nc.vector.tensor_add(o_fin[:, hs], o_sc[:, hs], osb[k][:, hs])
                nc.sync.dma_start(out[(k - 1) * 128:k * 128, :], o_fin[:])
    if split:
        _split_multi_waits(nc)
    return nc


def _masks():
    """Transposed boundary masks [256 window rows, 128 query cols], bf16."""
    j = np.arange(256)[:, None]
    i = np.arange(128)[None, :]
    bf = ml_dtypes.bfloat16
    maskT0 = np.where(j <= i, 0.0, MASKVAL).astype(bf)          # parity 0
    maskT1 = np.where(j <= 128 + i, 0.0, MASKVAL).astype(bf)    # parity 1
    return maskT0, maskT1


def _in_maps(x, w_q, w_k, w_v):
    bf = ml_dtypes.bfloat16
    x = np.asarray(x, np.float32)
    wqT = np.ascontiguousarray(np.asarray(w_q, np.float32).T).astype(bf)
    wkT = np.ascontiguousarray(np.asarray(w_k, np.float32).T).astype(bf)
    wvT = np.ascontiguousarray(np.asarray(w_v, np.float32).T).astype(bf)
    maskT0, maskT1 = _masks()
    ones = np.ones((128, 1), dtype=bf)

    in_maps = []
    for c in range(NCORES):
        b, p = divmod(c, 2)
        xb = x[b]                                    # [S, E]
        xoT = np.ascontiguousarray(xb[p * HT:(p + 1) * HT, :].T).astype(bf)
        qrows = np.concatenate(
            [xb[128 * (2 * (k - 1) + p):128 * (2 * (k - 1) + p) + 128, :]
             for k in range(1, NSLOT + 1)], axis=0)  # [NQ, E]
        xqT = np.ascontiguousarray(qrows.T).astype(bf)
        in_maps.append({
            "xoT": xoT, "xqT": xqT,
            "wqT": wqT, "wkT": wkT, "wvT": wvT,
            "maskT": maskT0 if p == 0 else maskT1,
            "ones": ones,
        })
    return in_maps


def _scatter(per_core_out):
    out = np.empty((B, S, D), dtype=np.float32)
    for c in range(NCORES):
        b, p = divmod(c, 2)
        oc = per_core_out[c]                         # [NQ, D]
        for k in range(1, NSLOT + 1):
            g = 2 * (k - 1) + p
            out[b, 128 * g:128 * (g + 1), :] = oc[128 * (k - 1):128 * k, :]
    return out


def kernel(x, w_q, w_k, w_v):
    global _prog
    if _prog is None:
        _prog = _build()
    in_maps = _in_maps(x, w_q, w_k, w_v)
    res = run_bass_kernel_spmd(_prog, in_maps, list(range(NCORES)))
    return _scatter([res.results[c]["out"] for c in range(NCORES)])


# revision 5
# speedup vs baseline: 1.0116x; 1.0116x over previous
"""Causal single-head attention on 8 Trainium2 NeuronCores, K/V pair-exchange.

Problem: x [4, 2048, 1024], w_q/w_k/w_v [1024, 1024] (nn.Linear convention,
y = x @ W.T). Computes q,k,v projections, causal softmax(q k^T / sqrt(D)) @ v.

Sharding: 2 cores per batch element. Core parity p owns token half
H_p = [p*1024, (p+1)*1024) and computes K^T/V for ONLY its half; halves are
exchanged between the pair via four 1MB AllGathers (replica groups [[0,1],
[2,3],[4,5],[6,7]]): K^T for own tokens [0:512], K^T [512:1024], V [0:512],
V [512:1024]. The two V AGs are fenced behind the two K AGs by
data-dependency rows in their bounce buffers (2 concurrent pair-AGs are
safe, 3+ corrupt the odd member - measured in a previous session).
Queries: parity-interleaved 128-tiles (slot k has a kv window of 256k
tokens), host-gathered, projected to Q^T kept in SBUF. Q^T is projected
LAST so it fills the PE while the K AGs are in flight.

Attention computes S^T (scores transposed, token-chunk-major) so the
softmax exp output IS the P^T layout the AV matmul needs - no transposes.
One token chunk c serves every slot k >= floor(c/2)+1 and those slots'
query columns are contiguous in Q^T, so each chunk's scores are just 1-2
wide matmuls per d-chunk instead of one per slot. exp reads straight from
PSUM (no max subtraction - scores/sqrt(D) are O(1) so exp cannot
overflow); per-slot row sums come from accumulating ones-matmuls; the
causal mask (each chunk is the boundary of exactly its first slot column
block) is folded in as one extra accumulation matmul of identity @ maskT.
AV runs as two passes: first chunks carried by the V-a AllGather, then V-b
chunks, so the last AG can arrive late without stalling the PE queue.

All matmul operands are bf16; softmax statistics and PSUM stay f32.
Every DMA is a 2D [128, W] transfer with a contiguous DRAM slab -
strided 3D DMAs generate descriptors on the triggering engine at ~1us/KB
of run-fragmentation and are a trap (measured).
"""
import numpy as np
import ml_dtypes
from contextlib import ExitStack

import concourse.bass as bass
import concourse.tile as tile
import concourse.mybir as mybir
from concourse.bass_utils import run_bass_kernel_spmd
from concourse.masks import make_identity

F32 = mybir.dt.float32
BF16 = mybir.dt.bfloat16
AF = mybir.ActivationFunctionType
AX = mybir.AxisListType

B, S, E, D = 4, 2048, 1024, 1024
NCORES = 8
NSLOT = 8              # slots k=1..8, kv window = 256*k tokens
NQ = NSLOT * 128       # queries per core
HT = S // 2            # tokens projected per core (own half)
HH = HT // 2           # token quarter (AG granularity)
EC = E // 128          # e-chunks
DC = D // 128          # d-chunks
NCH = S // 128         # token chunks
SCALE = 1.0 / 32.0     # 1/sqrt(D)
MASKVAL = -30000.0
GROUPS = [[0, 1], [2, 3], [4, 5], [6, 7]]

_prog = None


def _kmin(c):
    """First slot whose kv window includes token chunk c."""
    return c // 2 + 1


def _split_multi_waits(nc, max_waits=1):
    """The walrus build in this container has one sync-wait slot per
    instruction; hoist extra waits onto preceding same-engine NoOps."""
    n = 0
    for f in nc.m.functions:
        for b in f.blocks:
            insts = b.instructions
            out = []
            changed = False
            for ins in insts:
                si = ins.sync_info
                if si is not None and len(si.on_wait) > max_waits:
                    waits = list(si.on_wait)
                    for w in waits[:-max_waits]:
                        nop = mybir.InstNoOp(name=f"I-waitsplit-{n}")
                        n += 1
                        nop.engine = ins.engine
                        nop.sync_info = mybir.SyncInfo(on_wait=[w], on_update=[])
                        out.append(nop)
                    ins.sync_info = mybir.SyncInfo(
                        on_wait=waits[-max_waits:], on_update=list(si.on_update))
                    changed = True
                out.append(ins)
            if changed:
                b.instructions = out
    return nc


def _build(split=True):
    nc = bass.Bass(trn_type="TRN2", target_bir_lowering=False, debug=False)
    xoT = nc.dram_tensor("xoT", [E, HT], BF16, kind="ExternalInput").ap()
    xqT = nc.dram_tensor("xqT", [E, NQ], BF16, kind="ExternalInput").ap()
    wqT = nc.dram_tensor("wqT", [E, D], BF16, kind="ExternalInput").ap()
    wkT = nc.dram_tensor("wkT", [E, D], BF16, kind="ExternalInput").ap()
    wvT = nc.dram_tensor("wvT", [E, D], BF16, kind="ExternalInput").ap()
    # maskT: transposed causal boundary mask [2*128 window rows, 128 queries]
    maskin = nc.dram_tensor("maskT", [256, 128], BF16, kind="ExternalInput").ap()
    onesin = nc.dram_tensor("ones", [128, 1], BF16, kind="ExternalInput").ap()
    out = nc.dram_tensor("out", [NQ, D], F32, kind="ExternalOutput").ap()

    bncK, gathK = [], []
    for g in range(2):
        bncK.append(nc.dram_tensor(f"bncK{g}", [DC, 128, HH], BF16).ap())
        gathK.append(nc.dram_tensor(f"gathK{g}", [2, DC, 128, HH], BF16).ap())
    bncV, gathV = [], []
    for v in range(2):
        bncV.append(nc.dram_tensor(f"bncV{v}", [HH + 1, D], BF16).ap())
        gathV.append(nc.dram_tensor(f"gathV{v}", [2, HH + 1, D], BF16).ap())

    with tile.TileContext(nc) as tc, ExitStack() as ctx:
        const = ctx.enter_context(tc.tile_pool(name="const", bufs=1))
        ident = const.tile([128, 128], BF16)
        make_identity(nc, ident[:])
        maskT = const.tile([128, 256], BF16)   # [:, 0:128]=rows 0:128, etc
        nc.scalar.dma_start(maskT[:, 0:128], maskin[0:128, :])
        nc.scalar.dma_start(maskT[:, 128:256], maskin[128:256, :])
        ones = const.tile([128, 1], BF16)
        nc.scalar.dma_start(ones[:], onesin[:])

        # Q^T stays resident until the end of attention. col = d*NQ + q
        qtp = ctx.enter_context(tc.tile_pool(name="qtp", bufs=1))
        qts = qtp.tile([128, DC * NQ], BF16, name="qts")

        # ---- Phase 1: K_own^T -> AGs (by token half), V_own -> AGs, Q^T ----
        with tc.tile_pool(name="wp", bufs=1) as wp, \
             tc.tile_pool(name="xp", bufs=1) as xp, \
             tc.tile_pool(name="st", bufs=1) as stp, \
             tc.tile_pool(name="ps1", bufs=4, space="PSUM") as pp:
            # fused weight/activation tiles: col = e*width + c
            wk = wp.tile([128, EC * D], BF16, name="wk")
            wv = wp.tile([128, EC * D], BF16, name="wv")
            wq = wp.tile([128, EC * D], BF16, name="wq")
            xo = xp.tile([128, EC * HT], BF16, name="xo")
            xq = xp.tile([128, EC * NQ], BF16, name="xq")

            # startup: per-e 2D loads; first e-chunk of wk/xo first so the
            # first matmul can start ~2us after the queue drains to it
            for e in range(EC):
                nc.sync.dma_start(wk[:, e * D:e * D + 512],
                                  wkT[e * 128:(e + 1) * 128, :512])
                nc.sync.dma_start(xo[:, e * HT:(e + 1) * HT],
                                  xoT[e * 128:(e + 1) * 128, :])
            for e in range(EC):
                nc.sync.dma_start(wk[:, e * D + 512:(e + 1) * D],
                                  wkT[e * 128:(e + 1) * 128, 512:])
            for e in range(EC):
                nc.sync.dma_start(wv[:, e * D:(e + 1) * D],
                                  wvT[e * 128:(e + 1) * 128, :])
            for e in range(EC):
                nc.sync.dma_start(wq[:, e * D:(e + 1) * D],
                                  wqT[e * 128:(e + 1) * 128, :])
            for e in range(EC):
                nc.sync.dma_start(xq[:, e * NQ:(e + 1) * NQ],
                                  xqT[e * 128:(e + 1) * 128, :])

            # K_own^T: token-half-outer so AG-K0 (first 512 own tokens of
            # the pair) triggers after half of K_own. kown col = dd*HT + t.
            kown = stp.tile([128, DC * HT], BF16, name="kown")
            for g in range(2):
                for q in range(2):
                    psl = [pp.tile([128, 512], F32, name=f"pk{g}_{q}_{d}",
                                   tag="pp") for d in range(4)]
                    for e in range(EC):
                        for d in range(4):
                            dd = q * 4 + d
                            nc.tensor.matmul(
                                psl[d][:],
                                wk[:, e * D + dd * 128:e * D + (dd + 1) * 128],
                                xo[:, e * HT + g * 512:e * HT + (g + 1) * 512],
                                start=(e == 0), stop=(e == EC - 1))
                    for d in range(4):
                        dd = q * 4 + d
                        nc.vector.tensor_copy(
                            kown[:, dd * HT + g * 512:dd * HT + (g + 1) * 512],
                            psl[d][:])
                        nc.scalar.dma_start(
                            bncK[g][dd],
                            kown[:, dd * HT + g * 512:dd * HT + (g + 1) * 512])
                nc.gpsimd.collective_compute(
                    "AllGather", mybir.AluOpType.bypass, replica_groups=GROUPS,
                    ins=[bncK[g].opt()], outs=[gathK[g].opt()])

            # V_own: stationary x chunks, moving wv; token quarter v first.
            # vown col = t*D + c  (t = own-half token chunk 0..7)
            vown = stp.tile([128, (HT // 128) * D], BF16, name="vown")
            for v in range(2):
                for tl in range(HH // 128):
                    t = v * (HH // 128) + tl
                    for h in range(2):
                        ps = pp.tile([128, 512], F32, name=f"pv{t}_{h}", tag="pp")
                        for e in range(EC):
                            nc.tensor.matmul(
                                ps[:],
                                xo[:, e * HT + t * 128:e * HT + (t + 1) * 128],
                                wv[:, e * D + h * 512:e * D + (h + 1) * 512],
                                start=(e == 0), stop=(e == EC - 1))
                        nc.vector.tensor_copy(
                            vown[:, t * D + h * 512:t * D + (h + 1) * 512],
                            ps[:])
                    nc.scalar.dma_start(bncV[v][tl * 128:(tl + 1) * 128, :],
                                        vown[:, t * D:(t + 1) * D])
                # fence: the V AG may only trigger once the same-index K AG
                # has fully delivered (reads replica-1 bytes of its output)
                nc.scalar.dma_start(bncV[v][HH:HH + 1, 0:16],
                                    gathK[v][1, 0, 0:1, 0:16])
                nc.gpsimd.collective_compute(
                    "AllGather", mybir.AluOpType.bypass, replica_groups=GROUPS,
                    ins=[bncV[v].opt()], outs=[gathV[v].opt()])

            # Q^T last: fills the PE while the K AGs fly. col = d*NQ + q.
            for d in range(DC):
                for g in range(2):
                    ps = pp.tile([128, 512], F32, name=f"pq{d}_{g}", tag="pp")
                    for e in range(EC):
                        nc.tensor.matmul(
                            ps[:],
                            wq[:, e * D + d * 128:e * D + (d + 1) * 128],
                            xq[:, e * NQ + g * 512:e * NQ + (g + 1) * 512],
                            start=(e == 0), stop=(e == EC - 1))
                    nc.vector.tensor_copy(
                        qts[:, d * NQ + g * 512:d * NQ + (g + 1) * 512], ps[:])

        # ---- Phase 2: load gathered K^T / V into SBUF (2D DMAs) ----
        # kts col = d*S + t (global token order); vts col = t*D + c
        kvp = ctx.enter_context(tc.tile_pool(name="kvp", bufs=1))
        kts = kvp.tile([128, DC * S], BF16, name="kts")
        vts = kvp.tile([128, NCH * D], BF16, name="vts")
        for g in range(2):
            for r in range(2):
                base = r * HT + g * HH
                for d in range(DC):
                    eng = nc.sync if d % 2 == 0 else nc.scalar
                    eng.dma_start(kts[:, d * S + base:d * S + base + HH],
                                  gathK[g][r, d])
        for v in range(2):
            for r in range(2):
                for tl in range(HH // 128):
                    t = r * 8 + v * 4 + tl   # global chunk
                    eng = nc.sync if tl % 2 == 0 else nc.scalar
                    eng.dma_start(vts[:, t * D:(t + 1) * D],
                                  gathV[v][r, tl * 128:(tl + 1) * 128, :])

        # ---- Phase 3: chunk-major S^T scores + softmax (P^T straight) ----
        att = ctx.enter_context(tc.tile_pool(name="att", bufs=1))
        stats = ctx.enter_context(tc.tile_pool(name="stats", bufs=1))
        linv = stats.tile([128, NSLOT], F32, name="linv")
        # per-chunk P^T tiles: cols = slots kmin(c)..8, 128 each
        pT = {c: att.tile([128, 128 * (NSLOT + 1 - _kmin(c))], BF16,
                          name=f"pT{c}") for c in range(NCH)}
        osb = {k: att.tile([128, D], F32, name=f"osb{k}")
               for k in range(3, NSLOT + 1)}
        av_a = {k: [c for c in range(2 * k) if c % 8 < 4]
                for k in range(1, NSLOT + 1)}
        av_b = {k: [c for c in range(2 * k) if c % 8 >= 4]
                for k in range(1, NSLOT + 1)}

        with tc.tile_pool(name="ps3", bufs=1, space="PSUM") as pp3:
            ls = pp3.tile([128, 2], F32, name="ls", tag="lsp", bufs=1)

            def emit_lsum(k):
                for ci, c in enumerate(range(2 * k)):
                    j = k - _kmin(c)
                    nc.tensor.matmul(ls[:, 0:1],
                                     pT[c][:, j * 128:(j + 1) * 128],
                                     ones[:], start=(ci == 0),
                                     stop=(ci == 2 * k - 1))
                nc.vector.reciprocal(linv[:, k - 1:k], ls[:, 0:1])

            for c in range(NCH):
                km = _kmin(c)
                w = 128 * (NSLOT + 1 - km)
                npc = (w + 511) // 512
                sT = [pp3.tile([128, 512], F32, name=f"sT{c}_{i}", tag="sps",
                               bufs=3) for i in range(npc)]
                for i in range(npc):
                    pw = min(512, w - i * 512)
                    qoff = (km - 1) * 128 + i * 512
                    msk = (i == 0)
                    for d in range(DC):
                        nc.tensor.matmul(
                            sT[i][:, :pw],
                            kts[:, d * S + c * 128:d * S + (c + 1) * 128],
                            qts[:, d * NQ + qoff:d * NQ + qoff + pw],
                            start=(d == 0), stop=(d == DC - 1 and not msk))
                    if msk:
                        # chunk c is the causal boundary of slot kmin(c),
                        # which owns this chunk's first 128 query columns
                        mo = 0 if c % 2 == 0 else 128
                        nc.tensor.matmul(sT[i][:, 0:128], ident[:],
                                         maskT[:, mo:mo + 128],
                                         start=False, stop=True,
                                         skip_group_check=True)
                    nc.scalar.activation(pT[c][:, i * 512:i * 512 + pw],
                                         sT[i][:, :pw], AF.Exp, scale=SCALE)
                # slot k's last chunk is 2k-1; emit its row-sum matmuls one
                # chunk later so the PE never waits on the exp it just fed
                if c >= 2 and c % 2 == 0:
                    emit_lsum(c // 2)
            emit_lsum(NSLOT)

            # ---- Phase 4a: AV over the V-a chunks for every slot ----
            for k in range(1, NSLOT + 1):
                ca = av_a[k]
                o_ps = [pp3.tile([128, 512], F32, name=f"oa{k}_{h}", tag="ops",
                                 bufs=4) for h in range(2)]
                for ci, c in enumerate(ca):
                    j = k - _kmin(c)
                    for h in range(2):
                        nc.tensor.matmul(o_ps[h][:],
                                         pT[c][:, j * 128:(j + 1) * 128],
                                         vts[:, c * D + h * 512:c * D + (h + 1) * 512],
                                         start=(ci == 0), stop=(ci == len(ca) - 1))
                if not av_b[k]:
                    # k=1,2: fully V-a resident; normalize and write out
                    o_fin = att.tile([128, D], F32, name=f"ofa{k}", tag="ofin",
                                     bufs=2)
                    for h in range(2):
                        nc.scalar.activation(o_fin[:, h * 512:(h + 1) * 512],
                                             o_ps[h][:], AF.Copy,
                                             scale=linv[:, k - 1:k])
                    nc.sync.dma_start(out[(k - 1) * 128:k * 128, :], o_fin[:])
                else:
                    # stash normalized partial; V-b part added in phase 4b
                    for h in range(2):
                        nc.scalar.activation(osb[k][:, h * 512:(h + 1) * 512],
                                             o_ps[h][:], AF.Copy,
                                             scale=linv[:, k - 1:k])

            # ---- Phase 4b: AV over the V-b chunks, add, write out ----
            for k in range(3, NSLOT + 1):
                cb = av_b[k]
                o_ps = [pp3.tile([128, 512], F32, name=f"ob{k}_{h}", tag="ops",
                                 bufs=4) for h in range(2)]
                for ci, c in enumerate(cb):
                    j = k - _kmin(c)
                    for h in range(2):
                        nc.tensor.matmul(o_ps[h][:],
                                         pT[c][:, j * 128:(j + 1) * 128],
                                         vts[:, c * D + h * 512:c * D + (h + 1) * 512],
                                         start=(ci == 0), stop=(ci == len(cb) - 1))
                o_fin = att.tile([128, D], F32, name=f"ofb{k}", tag="ofin",
                                 bufs=2)
                o_sc = att.tile([128, D], F32, name=f"osc{k}", tag="osc", bufs=2)
                for h in range(2):
                    hs = slice(h * 512, (h + 1) * 512)
                    nc.scalar.activation(o_sc[:, hs], o_ps[h][:], AF.Copy,
                                         scale=linv[:, k - 1:k])
                    nc.vector.tensor_add(o_fin[:, hs], o_sc[:, hs], osb[k][:, hs])
                nc.sync.dma_start(out[(k - 1) * 128:k * 128, :], o_fin[:])
    if split:
        _split_multi_waits(nc)
    return nc


def _masks():
    """Transposed boundary masks [256 window rows, 128 query cols], bf16."""
    j = np.arange(256)[:, None]
    i = np.arange(128)[None, :]
    bf = ml_dtypes.bfloat16
    maskT0 = np.where(j <= i, 0.0, MASKVAL).astype(bf)          # parity 0
    maskT1 = np.where(j <= 128 + i, 0.0, MASKVAL).astype(bf)    # parity 1
    return maskT0, maskT1


def _in_maps(x, w_q, w_k, w_v):
    bf = ml_dtypes.bfloat16
    x = np.asarray(x, np.float32)
    wqT = np.ascontiguousarray(np.asarray(w_q, np.float32).T).astype(bf)
    wkT = np.ascontiguousarray(np.asarray(w_k, np.float32).T).astype(bf)
    wvT = np.ascontiguousarray(np.asarray(w_v, np.float32).T).astype(bf)
    maskT0, maskT1 = _masks()
    ones = np.ones((128, 1), dtype=bf)

    in_maps = []
    for c in range(NCORES):
        b, p = divmod(c, 2)
        xb = x[b]                                    # [S, E]
        xoT = np.ascontiguousarray(xb[p * HT:(p + 1) * HT, :].T).astype(bf)
        qrows = np.concatenate(
            [xb[128 * (2 * (k - 1) + p):128 * (2 * (k - 1) + p) + 128, :]
             for k in range(1, NSLOT + 1)], axis=0)  # [NQ, E]
        xqT = np.ascontiguousarray(qrows.T).astype(bf)
        in_maps.append({
            "xoT": xoT, "xqT": xqT,
            "wqT": wqT, "wkT": wkT, "wvT": wvT,
            "maskT": maskT0 if p == 0 else maskT1,
            "ones": ones,
        })
    return in_maps


def _scatter(per_core_out):
    out = np.empty((B, S, D), dtype=np.float32)
    for c in range(NCORES):
        b, p = divmod(c, 2)
        oc = per_core_out[c]                         # [NQ, D]
        for k in range(1, NSLOT + 1):
            g = 2 * (k - 1) + p
            out[b, 128 * g:128 * (g + 1), :] = oc[128 * (k - 1):128 * k, :]
    return out


def kernel(x, w_q, w_k, w_v):
    global _prog
    if _prog is None:
        _prog = _build()
    in_maps = _in_maps(x, w_q, w_k, w_v)
    res = run_bass_kernel_spmd(_prog, in_maps, list(range(NCORES)))
    return _scatter([res.results[c]["out"] for c in range(NCORES)])
